# revision 25
# baseline (speedup 1.0000x reference)
"""Causal self-attention on 8 TRN2 NeuronCores.

Problem: x[2,2048,1024], wq/wk/wv/wo[1024,1024] (nn.Linear convention,
out = y @ W.T), H=16 heads, D=64, causal softmax, f32.

Sharding: tensor-parallel over heads x data-parallel over batch.
Core i handles batch b=i//4 and head group g=i%4 (4 heads each).
wq/wk/wv are split row-wise (output-feature) per head group; wo is
split column-wise; each core returns a partial output projection
out_partial[b] and the host sums the 4 partials per batch.

On-device layout is fully "feature-major" (transposed): the host passes
xT=x[b].T etc so every matmul sees its contraction dim on SBUF
partitions and no on-device transposes are needed. Attention uses the
S^T formulation: per key-chunk ki, scores^T[k, q] for all valid query
spans land in wide multi-bank PSUM tiles so ScalarE exp runs in wide
strokes (its (N+352)/1.2ns cost is per call); causal masking is a
multiplicative bf16 triangular mask on the P^T tile after exp (the
fully-invalid region is simply never computed or read); the softmax
row sums ride a ones-column folded into the PV matmul's stationary
operand; the 1/sum normalization transposes the sums via a permuted
DRAM-roundtrip DMA so DVE reciprocal runs on 128 partitions instead
of one. Projections and scores run in float32r (TF32-like, 1
cycle/row at N>=256); P^T and V in bf16 with fp32 PSUM accumulation.
"""

import sys

for _p in ("/opt/trn_rl_repo", "/root/.axon_site"):
    if _p not in sys.path:
        sys.path.insert(0, _p)

import numpy as np

import concourse.bass as bass
import concourse.mybir as mybir
import concourse.tile as tile
from concourse import bacc
from concourse.bass_utils import run_bass_kernel_spmd

B, T, C, H = 2, 2048, 1024, 16
DH = C // H            # 64 head dim
HG = 4                 # heads per core
GW = HG * DH           # 256 features per head group
NB = T // 128          # 16 key chunks
NS = T // 512          # 4 query spans
KC = C // 128          # 8 contraction chunks over C
SCALE = 1.0 / float(np.sqrt(DH))
N_CORES = 8

F32 = mybir.dt.float32
F32R = mybir.dt.float32r
BF16 = mybir.dt.bfloat16
EXP = mybir.ActivationFunctionType.Exp
COPY = mybir.ActivationFunctionType.Copy


def build_nc():
    nc = bacc.Bacc("TRN2", target_bir_lowering=False, debug=False,
                   num_devices=N_CORES)
    xT = nc.declare_dram_parameter("xT", [C, T], F32R, isOutput=False)
    wqT = nc.declare_dram_parameter("wqT", [C, GW], F32R, isOutput=False)
    wkT = nc.declare_dram_parameter("wkT", [C, GW], F32R, isOutput=False)
    wvT = nc.declare_dram_parameter("wvT", [C, GW], F32R, isOutput=False)
    woT = nc.declare_dram_parameter("woT", [GW, C], F32R, isOutput=False)
    outT = nc.declare_dram_parameter("outT", [C, T], F32, isOutput=True)
    s_dram = nc.dram_tensor("s_scratch", [HG, NS, 512], F32)
    r_dram = nc.dram_tensor("r_scratch", [HG, NS, 512], F32)

    with tile.TileContext(nc) as tc:
        with tc.tile_pool(name="pers", bufs=1) as pers:
            # ---- persistent SBUF tensors; DMAs issued in consumption
            # order so the first projection matmuls start early ----
            wk_t = [pers.tile([128, GW], F32R, tag=f"wk{i}", name=f"wk{i}")
                    for i in range(KC)]
            for i in range(KC):
                nc.sync.dma_start(out=wk_t[i], in_=wkT[i * 128:(i + 1) * 128, :])
            xts = [pers.tile([128, T], F32R, tag=f"xT{i}", name=f"xT{i}")
                   for i in range(KC)]
            wq_t = [pers.tile([128, GW], F32R, tag=f"wq{i}", name=f"wq{i}")
                    for i in range(KC)]
            for s in range(NS):          # span-sized sub-loads
                for i in range(KC):
                    nc.sync.dma_start(
                        out=xts[i][:, s * 512:(s + 1) * 512],
                        in_=xT[i * 128:(i + 1) * 128, s * 512:(s + 1) * 512])
                if s == 0:
                    for i in range(KC):
                        nc.sync.dma_start(
                            out=wq_t[i], in_=wqT[i * 128:(i + 1) * 128, :])
            wv_t = [pers.tile([128, GW], F32R, tag=f"wv{i}", name=f"wv{i}")
                    for i in range(KC)]
            for i in range(KC):
                nc.sync.dma_start(out=wv_t[i], in_=wvT[i * 128:(i + 1) * 128, :])
            wo_t = [pers.tile([128, C], F32R, tag=f"wo{j}", name=f"wo{j}")
                    for j in range(2)]
            for j in range(2):
                nc.sync.dma_start(out=wo_t[j], in_=woT[j * 128:(j + 1) * 128, :])

            qts = [pers.tile([128, T], F32R, tag=f"qT{m}", name=f"qT{m}") for m in range(2)]
            kts = [pers.tile([128, T], F32R, tag=f"kT{m}", name=f"kT{m}") for m in range(2)]
            yts = [pers.tile([128, T], F32R, tag=f"yT{m}", name=f"yT{m}") for m in range(2)]

            # bf16 triangular mask for the diagonal 128x128 strip of
            # P^T: keep (1) where col >= row i.e. q >= k, else 0
            trim = pers.tile([128, 128], BF16, tag="trim", name="trim")
            nc.gpsimd.memset(trim, 1.0)
            nc.gpsimd.affine_select(
                out=trim, in_=trim, compare_op=mybir.AluOpType.is_ge,
                fill=0.0, base=0, pattern=[[1, 128]], channel_multiplier=-1)
            # ones [128, 4] in bf16 for V's ones-columns
            ones4 = pers.tile([128, 4], BF16, tag="ones4", name="ones4")
            for j in range(4):
                nc.scalar.activation(
                    out=ones4[:, j:j + 1],
                    in_=nc.const_aps.tensor(1.0, [128, 1]), func=COPY)

            # ---- phase 1: projections ----
            vts = [pers.tile([128, HG * 65], BF16, tag=f"V{tb}", name=f"V{tb}")
                   for tb in range(NB)]
            with tc.tile_pool(name="pp1", bufs=6, space="PSUM") as pp1, \
                 tc.tile_pool(name="pp2", bufs=2, space="PSUM") as pp2:
                # kT and qT first (attention consumes them earliest);
                # span-outer so each 8-matmul group needs only one
                # span's worth of xT in SBUF
                for wt, dest in ((wk_t, kts), (wq_t, qts)):
                    for m in range(2):
                        for s in range(NS):
                            ps = pp1.tile([128, 512], F32, tag="projps",
                                          name="projps")
                            for k in range(KC):
                                nc.tensor.matmul(
                                    ps,
                                    wt[k][:, m * 128:(m + 1) * 128],
                                    xts[k][:, s * 512:(s + 1) * 512],
                                    start=(k == 0), stop=(k == KC - 1))
                            nc.vector.tensor_copy(
                                out=dest[m][:, s * 512:(s + 1) * 512],
                                in_=ps)
                # V in natural [t, d] layout: stationary = xT chunk
                for tb in range(NB):
                    vps = pp2.tile([128, GW], F32, tag="vps", name="vps")
                    for k in range(KC):
                        nc.tensor.matmul(
                            vps, xts[k][:, tb * 128:(tb + 1) * 128], wv_t[k],
                            start=(k == 0), stop=(k == KC - 1))
                    vt = vts[tb]
                    for h in range(HG):
                        nc.vector.tensor_copy(
                            out=vt[:, h * 65:h * 65 + 64],
                            in_=vps[:, h * 64:(h + 1) * 64])
                    nc.vector.tensor_copy(
                        out=vt.rearrange("p (h c) -> p h c", c=65)[:, :, 64],
                        in_=ones4)

            # ---- phase 2: attention; per head, key-chunk-outer so exp
            # runs in wide multi-span strokes ----
            with tc.tile_pool(name="mgs", bufs=4, space="PSUM") as mgs, \
                 tc.tile_pool(name="pvs", bufs=1, space="PSUM") as pvs, \
                 tc.tile_pool(name="ptp", bufs=8) as ptp, \
                 tc.tile_pool(name="rp", bufs=4) as rp:
                for h in range(HG):
                    qt, kt, yt = qts[h // 2], kts[h // 2], yts[h // 2]
                    po = (h % 2) * 64
                    pv = [pvs.tile([65, 512], F32, tag=f"pv{s}",
                                   name=f"pv{s}") for s in range(NS)]
                    for ki in range(NB):
                        smin = ki // 4
                        j = ki % 4
                        nsp = NS - smin          # spans covered by this ki
                        c0 = 128 * j             # first valid col in span smin
                        # one [128,512] score tile per span: a 4-deep
                        # ring decouples PE from ScalarE twice as far as
                        # the previous 2x[128,1024] layout (same 4 banks)
                        pts = []
                        for t2 in range(nsp):
                            s = smin + t2
                            cc = c0 if t2 == 0 else 0
                            mg = mgs.tile([128, 512], F32, tag="mg",
                                          name="mg")
                            nc.tensor.matmul(
                                mg[:, cc:],
                                kt[po:po + 64, ki * 128:(ki + 1) * 128],
                                qt[po:po + 64,
                                   s * 512 + cc:(s + 1) * 512],
                                start=True, stop=True)
                            pt = ptp.tile([128, 512], BF16, tag="pt",
                                          name="pt")
                            nc.scalar.activation(
                                out=pt[:, cc:], in_=mg[:, cc:],
                                func=EXP, scale=SCALE)
                            pts.append(pt)
                        # causal mask on the diagonal strip (span smin)
                        nc.vector.tensor_mul(
                            out=pts[0][:, c0:c0 + 128],
                            in0=pts[0][:, c0:c0 + 128], in1=trim)
                        # PV accumulation per span
                        done = []
                        for t2 in range(nsp):
                            s = smin + t2
                            cc = c0 if t2 == 0 else 0
                            nc.tensor.matmul(
                                pv[s][:, cc:],
                                vts[ki][:, h * 65:(h + 1) * 65],
                                pts[t2][:, cc:],
                                start=(ki == 0), stop=(ki == 4 * s + 3))
                            if ki == 4 * s + 3:
                                done.append(s)
                        # finalize completed spans immediately: free the
                        # PV bank fast, then normalize via a
                        # DMA-transposed reciprocal off the critical path
                        for s in done:
                            yv = rp.tile([65, 512], F32, tag="yv", name="yv")
                            nc.vector.tensor_copy(out=yv, in_=pv[s][0:65, :])
                            nc.gpsimd.dma_start(out=s_dram[h, s, :],
                                                in_=yv[64:65, :])
                            st = rp.tile([128, 4], F32, tag="st", name="st")
                            nc.gpsimd.dma_start(
                                out=st,
                                in_=s_dram[h, s, :].rearrange("(c p) -> p c", p=128))
                            rts = rp.tile([128, 4], F32, tag="rts", name="rts")
                            nc.vector.reciprocal(out=rts, in_=st)
                            nc.gpsimd.dma_start(
                                out=r_dram[h, s, :].rearrange("(c p) -> p c", p=128),
                                in_=rts)
                            rb = rp.tile([64, 512], F32, tag="rb", name="rb")
                            rsl = r_dram[h, s, :]
                            nc.gpsimd.dma_start(
                                out=rb,
                                in_=bass.AP(tensor=rsl.tensor, offset=rsl.offset,
                                            ap=[[0, 64]] + list(rsl.ap)))
                            nc.vector.tensor_mul(
                                out=yt[po:po + 64, s * 512:(s + 1) * 512],
                                in0=yv[0:64, :], in1=rb)

            # ---- phase 3: output projection (partial sums) ----
            with tc.tile_pool(name="ops", bufs=4, space="PSUM") as ops, \
                 tc.tile_pool(name="ost", bufs=4) as ost:
                for m in range(8):
                    for s in range(NS):
                        op = ops.tile([128, 512], F32, tag="op", name="op")
                        for j in range(2):
                            nc.tensor.matmul(
                                op,
                                wo_t[j][:, m * 128:(m + 1) * 128],
                                yts[j][:, s * 512:(s + 1) * 512],
                                start=(j == 0), stop=(j == 1))
                        ot = ost.tile([128, 512], F32, tag="ot", name="ot")
                        nc.vector.tensor_copy(out=ot, in_=op)
                        nc.sync.dma_start(
                            out=outT[m * 128:(m + 1) * 128,
                                     s * 512:(s + 1) * 512],
                            in_=ot)
    nc.compile()
    return nc


_NC_CACHE = None


def _get_nc():
    global _NC_CACHE
    if _NC_CACHE is None:
        _NC_CACHE = build_nc()
    return _NC_CACHE


def make_in_maps(x, wq, wk, wv, wo):
    x = np.asarray(x, dtype=np.float32)
    wq = np.asarray(wq, dtype=np.float32)
    wk = np.asarray(wk, dtype=np.float32)
    wv = np.asarray(wv, dtype=np.float32)
    wo = np.asarray(wo, dtype=np.float32)
    in_maps = []
    for core in range(N_CORES):
        b, g = core // HG, core % HG
        rows = slice(g * GW, (g + 1) * GW)
        in_maps.append({
            "xT": np.ascontiguousarray(x[b].T),
            "wqT": np.ascontiguousarray(wq[rows, :].T),
            "wkT": np.ascontiguousarray(wk[rows, :].T),
            "wvT": np.ascontiguousarray(wv[rows, :].T),
            "woT": np.ascontiguousarray(wo[:, rows].T),
        })
    return in_maps


def run(x, wq, wk, wv, wo, trace=False, tmpdir=None):
    nc = _get_nc()
    in_maps = make_in_maps(x, wq, wk, wv, wo)
    res = run_bass_kernel_spmd(nc, in_maps, core_ids=list(range(N_CORES)),
                               trace=trace, tmpdir=tmpdir)
    out = np.zeros((B, T, C), dtype=np.float32)
    for core in range(N_CORES):
        out[core // HG] += res.results[core]["outT"].T
    return out, res


def kernel(x, wq, wk, wv, wo):
    out, _ = run(x, wq, wk, wv, wo)
    return out
